# revision 59
# baseline (speedup 1.0000x reference)
"""Trainium2 Bass kernel for nn_CausalSelfAttention_68496138437292.

Sharding: 8 cores = 2 batches x 4 head-groups; core c handles batch c//4 and
local heads [4*(c%4), 4*(c%4)+4).  The Tversky projection is sharded over
out_features (each core computes a 256-wide o-slice); the 16-wide feature
contraction x_f (summed over D, split across head-groups) uses one small
AllReduce per 512-token chunk over each batch's 4-core group.

Key structural ideas (vs the phase-serial f32r baseline):
- bf16 everywhere on the PE; all (head, half) channel blocks are packed 4 per
  128-partition tile in natural order, so projection PSUM drains straight into
  the attention layout with no DMA repacking.
- The attention OUTPUT is never materialized: the final output only needs
  x_f = feat . y^T, and  feat_slice . (P V / den)^T = (P (V W^T))^T / den,
  with  V W^T = x @ (w_v^T W^T)  folded into the projection as 128 extra
  channels (host-precomputed fold).  Attention per (head, half) reduces to
  scores -> exp -> one 17-wide PV matmul (16 feat cols + a ones column that
  emits the softmax denominator for free).
- Scores/PV use PE array tiling (tile_position) for ~1.4x matmul throughput;
  exp runs as [128, 1024] ACT ops over multi-bank PSUM with trimmed causal
  widths; rmsnorm rsqrt+gain is one ln + one exp (single ACT table set).
- Projection of chunk c+1 is interleaved into attention of chunk j=c so the
  PE keeps busy while ACT chews exp; Tversky tail is pipelined per chunk with
  one [16, 512] AllReduce each, its post-collective output step emitted last
  so no PE op ever queues behind collective latency.

Hard-won correctness rule: the tile framework does NOT track DRAM RAW
dependencies (MANAGED_SPACES is SBUF/PSUM only).  Every DRAM round-trip
(scale broadcast staging, reciprocal broadcast staging) must keep its write
and read on the SAME DMA queue, and collective inputs/outputs must be
written/read on the gpsimd queue the collective itself occupies -- otherwise
the kernel NaNs intermittently under scheduling jitter.

Measured on 8x axon trn2: ~305 us (vs 850 us baseline), rel err 5.8e-3.
"""

import math
from contextlib import ExitStack

import ml_dtypes
import numpy as np

import concourse.bass as bass
import concourse.mybir as mybir
import concourse.tile as tile
from concourse import bacc
from concourse.bass_utils import run_bass_kernel_spmd

F32 = mybir.dt.float32
F32R = mybir.dt.float32r
BF16 = mybir.dt.bfloat16
AF = mybir.ActivationFunctionType
ALU = mybir.AluOpType

DIM, NH, HD = 1024, 16, 64
ROPE_BASE, TRAIN_LEN, YARN_MAX = 10000.0, 1024, 4096
GROUP = 64
EPS = 1e-05
B = 2
N_CORES = 8
HPC = 4          # heads per core
OSL = 256        # out-feature slice per core
NF = 16          # tversky feature count
PVW = 17         # PV rhs width: 16 feat cols + ones col

BF = ml_dtypes.bfloat16


# ----------------------------------------------------------------- host math

def _ternary_deq(w: np.ndarray) -> np.ndarray:
    bf = ml_dtypes.bfloat16
    wb = w.astype(bf)
    wg = wb.reshape(-1, GROUP)
    m = (np.sum(np.abs(wg), axis=-1, keepdims=True, dtype=np.float32) / GROUP).astype(bf)
    scale = np.maximum(m.astype(np.float32), np.float32(1e-8)).astype(bf)
    ratio = (wg.astype(np.float32) / scale.astype(np.float32)).astype(bf)
    q = np.clip(np.round(ratio.astype(np.float32)), -1.0, 1.0).astype(bf)
    deq = (q.astype(np.float32) * scale.astype(np.float32)).astype(bf)
    return deq.reshape(wb.shape).astype(np.float32)


def _rope_tables(seqlen: int):
    rd = HD
    ar = np.arange(0, rd, 2, dtype=np.float32)
    inv_freq = 1.0 / ROPE_BASE ** (ar / rd)
    scale = TRAIN_LEN / YARN_MAX
    ramp = np.clip((ar / rd - 0.25) / 0.75, 0.0, 1.0)
    inv_freq = inv_freq / (ramp * (1.0 / scale - 1.0) + 1.0)
    freqs = np.arange(seqlen, dtype=np.float32)[:, None] * inv_freq[None, :]
    return np.cos(freqs).T.astype(np.float32), np.sin(freqs).T.astype(np.float32)


def _sigmoid(x):
    return 1.0 / (1.0 + np.exp(-x))


# ------------------------------------------------------------ device program

def build_program(S: int, dbg: bool = False):
    NC = S // 512            # 512-token chunks
    NT = S // 128            # 128-token k tiles

    nc = bacc.Bacc("TRN2", target_bir_lowering=False, debug=False,
                   num_devices=N_CORES)

    # DRAM I/O
    d_xT = nc.dram_tensor("xT", [DIM, S], BF16, kind="ExternalInput")
    # 640 = 256 q + 256 k + 128 vw-fold channels
    d_wT = nc.dram_tensor("wT", [DIM, 640], BF16, kind="ExternalInput")
    d_cosF = nc.dram_tensor("cosF", [128, S], BF16, kind="ExternalInput")
    d_sinF = nc.dram_tensor("sinF", [128, S], BF16, kind="ExternalInput")
    d_tri4 = nc.dram_tensor("tri4", [128, 512], BF16, kind="ExternalInput")
    d_mask8 = nc.dram_tensor("mask8", [128, 32], BF16, kind="ExternalInput")
    d_smask = nc.dram_tensor("smask", [128, 16], BF16, kind="ExternalInput")
    d_lngain = nc.dram_tensor("lngain", [8, 1], F32, kind="ExternalInput")
    d_ident = nc.dram_tensor("ident", [128, 128], BF16, kind="ExternalInput")
    d_AT = nc.dram_tensor("AT", [16, OSL], BF16, kind="ExternalInput")
    d_BT = nc.dram_tensor("BT", [16, OSL], BF16, kind="ExternalInput")
    d_out = nc.dram_tensor("out", [OSL, S], F32, kind="ExternalOutput")
    if dbg:
        d_dbg_xf = nc.dram_tensor("dbg_xf", [16, S], F32, kind="ExternalOutput")
        d_dbg_xfar = nc.dram_tensor("dbg_xfar", [16, S], F32, kind="ExternalOutput")

    with tile.TileContext(nc) as tc:
        persist = ExitStack()
        cpool = persist.enter_context(tc.tile_pool(name="consts", bufs=1))
        qkpool = persist.enter_context(tc.tile_pool(name="qk", bufs=1))
        vwpool = persist.enter_context(tc.tile_pool(name="vwrhs", bufs=1))
        xfpool = persist.enter_context(tc.tile_pool(name="xft", bufs=1))
        wpool = persist.enter_context(tc.tile_pool(name="wts", bufs=1))
        drpool = persist.enter_context(
            tc.tile_pool(name="drscratch", bufs=1, space="DRAM"))

        # transient pools
        xpool = persist.enter_context(tc.tile_pool(name="xstream", bufs=8))
        sqpool = persist.enter_context(tc.tile_pool(name="sq", bufs=4))
        scpool = persist.enter_context(tc.tile_pool(name="scb", bufs=4))
        s8pool = persist.enter_context(tc.tile_pool(name="sc8", bufs=2))
        rppool = persist.enter_context(tc.tile_pool(name="ropetmp", bufs=2))
        espool = persist.enter_context(tc.tile_pool(name="es", bufs=6))
        tailp = persist.enter_context(tc.tile_pool(name="tail", bufs=2))

        # PSUM budget (8 banks): work 2 + waves 2x2 + pv 2 = 8
        work_ps = persist.enter_context(
            tc.tile_pool(name="work", bufs=2, space="PSUM"))
        wave_ps = persist.enter_context(
            tc.tile_pool(name="wave", bufs=2, space="PSUM"))
        pv_ps = persist.enter_context(
            tc.tile_pool(name="pv", bufs=1, space="PSUM"))

        def aux_tile():
            # shared double-buffered [128, 512] psum bank; callers slice it
            return work_ps.tile([128, 512], F32, tag="w", name="w")

        # ---- persistent SBUF ----
        # weights first: the first projection chain only needs wts + x(0)
        wts0 = wpool.tile([128, 640], BF16, name="w0")
        nc.sync.dma_start(wts0[:], d_wT[0:128, :])
        cosF = cpool.tile([128, S], BF16, name="cosF")
        sinF = cpool.tile([128, S], BF16, name="sinF")
        tri4 = cpool.tile([128, 512], BF16, name="tri4")
        mask8 = cpool.tile([128, 32], BF16, name="mask8")
        smask = cpool.tile([128, 16], BF16, name="smask")
        lngain = cpool.tile([8, 1], F32, name="lngain")
        eps8 = cpool.tile([8, 1], F32, name="eps8")
        nc.vector.memset(eps8[:], EPS)
        ident = cpool.tile([128, 128], BF16, name="ident")
        AT = cpool.tile([16, OSL], BF16, name="AT")
        BT = cpool.tile([16, OSL], BF16, name="BT")

        # Pre-load the exp+ln table set so the placement pass never needs to
        # thrash between exp_and_others / natural_log per chunk.
        tables = list(__import__("concourse.hw_specs", fromlist=["x"])
                      .get_activation_tables(nc.m.arch).keys())
        set_id = tables.index("natural_log_exp_and_others")
        nc.scalar.add_instruction(mybir.InstLoadActFuncSet(
            name=nc.get_next_instruction_name(), act_func_set_id=set_id,
            ins=[], outs=[]))

        wts = [wts0] + [wpool.tile([128, 640], BF16, name=f"w{d}")
                        for d in range(1, 8)]
        _wq = [nc.sync, nc.gpsimd, nc.sync]
        for d in range(1, 8):
            _wq[d % 3].dma_start(wts[d][:], d_wT[d * 128:(d + 1) * 128, :])

        def load_consts():
            # emitted after the first projection chain so the x/w DMAs lead
            nc.gpsimd.dma_start(mask8[:], d_mask8[:])
            nc.gpsimd.dma_start(lngain[:], d_lngain[:])
            nc.gpsimd.dma_start(cosF[:], d_cosF[:])
            nc.gpsimd.dma_start(sinF[:], d_sinF[:])
            nc.gpsimd.dma_start(tri4[:], d_tri4[:])
            nc.gpsimd.dma_start(smask[:], d_smask[:])
            nc.gpsimd.dma_start(ident[:], d_ident[:])
            nc.gpsimd.dma_start(AT[:], d_AT[:])
            nc.gpsimd.dma_start(BT[:], d_BT[:])

        qa = [qkpool.tile([128, S], BF16, name=f"qa{t}") for t in range(2)]
        ka = [qkpool.tile([128, S], BF16, name=f"ka{t}") for t in range(2)]
        # PV rhs per ktile: [k 128, 8 problems x 17]; ones col at 16 mod 17
        rhs_vw = [vwpool.tile([128, 8 * PVW], BF16, name=f"rvw{t}")
                  for t in range(NT)]
        for t in range(NT):
            nc.vector.memset(
                rhs_vw[t][:].rearrange("p (g c) -> p g c", c=PVW)[:, :, 16:17],
                1.0)
        # xf^T strips per tile-group: rows 32p..32p+16 = (xf contrib | den)
        xft = [xfpool.tile([128, S], F32, name=f"xft{t}") for t in range(2)]

        # ---------------- emission helpers ----------------

        def proj_chunk_steps(c):
            """Returns a list of closures emitting projection of chunk c."""
            s0 = c * 512
            steps = []
            xt = [None] * 8
            sq_t = [None] * 4
            stat = [None]
            sc8 = [None]

            def load_x():
                for d in range(8):
                    xt[d] = xpool.tile([128, 512], BF16, tag="xt", name="xt")
                    nc.gpsimd.dma_start(xt[d][:], d_xT[d * 128:(d + 1) * 128,
                                                       s0:s0 + 512])
            steps.append(load_x)

            # 4 qk chains: ot 0,1 = q tiles, ot 2,3 = k tiles
            def make_qk(ot):
                def f():
                    dst = qa[ot] if ot < 2 else ka[ot - 2]
                    pq = aux_tile()
                    for d in range(8):
                        nc.tensor.matmul(pq[:], wts[d][:, ot * 128:(ot + 1) * 128],
                                         xt[d][:], start=(d == 0), stop=(d == 7))
                    # unscaled drain on ACT (DVE is busy with rope; ACT drains
                    # promptly, so the next chain's WAW stall is short)
                    nc.scalar.activation(dst[:, s0:s0 + 512], pq[:], AF.Copy)
                    sq = sqpool.tile([128, 512], BF16, tag="sq", name="sq")
                    nc.vector.tensor_mul(sq[:], dst[:, s0:s0 + 512],
                                         dst[:, s0:s0 + 512])
                    sq_t[ot] = sq
                return f
            for ot in range(4):
                steps.append(make_qk(ot))

            def stats():
                st = aux_tile()
                for ot in range(4):
                    nc.tensor.matmul(st[0:8, :], mask8[:, ot * 8:(ot + 1) * 8],
                                     sq_t[ot][:], start=(ot == 0), stop=(ot == 3))
                lnt = s8pool.tile([8, 512], F32, tag="lnt", name="lnt")
                nc.scalar.activation(lnt[:], st[0:8, :], AF.Ln, scale=1.0 / HD,
                                     bias=eps8[:])
                s8 = s8pool.tile([8, 512], BF16, tag="sc8", name="sc8")
                nc.scalar.activation(s8[:], lnt[:], AF.Exp, scale=-0.5,
                                     bias=lngain[:])
                sc8[0] = s8
            steps.append(stats)

            def scale_rope():
                # broadcast row scales into block layout (via DRAM scratch --
                # SBUF sources cannot have a zero partition step), then
                # scale+rope
                dr8 = drpool.tile([8, 512], BF16, tag="dr8", bufs=2, name="dr8")
                nc.sync.dma_start(dr8[:], sc8[0][:])
                qs = [nc.sync, nc.sync, nc.sync, nc.sync]
                scb = []
                for tt in range(4):          # 2 q tiles then 2 k tiles
                    sb = scpool.tile([128, 512], BF16, tag="scb", name="scb")
                    for hh in range(2):
                        row = (tt // 2) * 4 + (tt % 2) * 2 + hh
                        qs[tt].dma_start(
                            sb[hh * 64:hh * 64 + 64, :],
                            dr8[row:row + 1, :].to_broadcast([64, 512]))
                    scb.append(sb)
                tiles = [qa[0], qa[1], ka[0], ka[1]]
                for tt in range(4):
                    nc.vector.tensor_mul(tiles[tt][:, s0:s0 + 512],
                                         tiles[tt][:, s0:s0 + 512], scb[tt][:])
                # rope: prefetch partner blocks (adjacent 32-row block), then
                # x = x*cos + prt*sinF (sinF carries the half sign)
                for tt in range(4):
                    prt = rppool.tile([128, 512], BF16, tag="prt", name="prt")
                    for p in range(4):
                        qs[(tt + p) % 4].dma_start(
                            prt[32 * p:32 * p + 32, :],
                            tiles[tt][32 * (p ^ 1):32 * (p ^ 1) + 32, s0:s0 + 512])
                    tb = rppool.tile([128, 512], BF16, tag="tb", name="tb")
                    nc.vector.tensor_mul(tb[:], prt[:], sinF[:, s0:s0 + 512])
                    nc.vector.tensor_mul(tiles[tt][:, s0:s0 + 512],
                                         tiles[tt][:, s0:s0 + 512],
                                         cosF[:, s0:s0 + 512])
                    nc.vector.tensor_add(tiles[tt][:, s0:s0 + 512],
                                         tiles[tt][:, s0:s0 + 512], tb[:])
            steps.append(scale_rope)

            def vw_chain():
                pv = aux_tile()
                for d in range(8):
                    nc.tensor.matmul(pv[:], wts[d][:, 512:640], xt[d][:],
                                     start=(d == 0), stop=(d == 7))
                vw_sb = sqpool.tile([128, 512], BF16, tag="vwsb", name="vwsb")
                nc.scalar.activation(vw_sb[:], pv[:], AF.Copy)
                # transpose each 128-token block: vw_sb [ch 128, s] -> [s, ch]
                for i in range(4):
                    pt = aux_tile()
                    nc.tensor.matmul(pt[:, 0:128], vw_sb[:, i * 128:(i + 1) * 128],
                                     ident[:], start=True, stop=True)
                    t = c * 4 + i
                    dst = rhs_vw[t][:].rearrange(
                        "p (g c) -> p g c", c=PVW)[:, :, 0:16]
                    nc.vector.tensor_copy(
                        dst, pt[:, 0:128].rearrange("p (g c) -> p g c", c=16))
            steps.append(vw_chain)
            return steps

        # attention state: per chunk j, accumulate xf strips in 2 psum banks
        def attn_chunk(j, inject):
            """Emit attention for q chunk j; call inject() between t-steps to
            interleave next chunk's projection work."""
            ntk = 4 * (j + 1)
            xfa = [pv_ps.tile([128, 512], F32, tag=f"xfa{qt}", name=f"xfa{qt}")
                   for qt in range(2)]
            es_q = {}
            LAG = 2

            def emit_pv(t):
                off = max(0, (t - 4 * j) * 128)
                for qt in range(2):
                    es = es_q.pop((t, qt))
                    es3 = es[:].rearrange("p (g c) -> p g c", c=512)
                    for p in range(4):
                        nc.tensor.matmul(
                            xfa[qt][32 * p:32 * p + PVW, off:512],
                            rhs_vw[t][:, (4 * qt + p) * PVW:(4 * qt + p + 1) * PVW],
                            es3[:, p, off:512],
                            start=(t == 0), stop=(t == ntk - 1),
                            tile_position=(0, 32 * p),
                            skip_group_check=True)

            for t in range(ntk):
                off = max(0, (t - 4 * j) * 128)
                w = 512 - off
                for qt in range(2):
                    es = espool.tile([128, 2048], BF16, tag="es", name="es")
                    for pair in range(2):
                        ps = wave_ps.tile([128, 1024], F32, tag="wv", name="wv")
                        for pp in range(2):
                            p = pair * 2 + pp
                            nc.tensor.matmul(
                                ps[:, pp * 512 + off:(pp + 1) * 512],
                                ka[qt][32 * p:32 * p + 32, t * 128:(t + 1) * 128],
                                qa[qt][32 * p:32 * p + 32,
                                       j * 512 + off:(j + 1) * 512],
                                start=True, stop=True,
                                tile_position=(32 * p, 0))
                        ps3 = ps[:].rearrange("p (g c) -> p g c", c=512)
                        es3 = es[:].rearrange("p (g c) -> p g c", c=512)
                        nc.scalar.activation(
                            es3[:, 2 * pair:2 * pair + 2, off:512],
                            ps3[:, :, off:512], AF.Exp)
                    if t >= 4 * j:
                        # causal mask on the diagonal 128-block of each strip
                        dv = es[:].rearrange("p (g c) -> p g c", c=512)[
                            :, :, off:off + 128]
                        tri = tri4[:].rearrange("p (g c) -> p g c", c=128)
                        nc.vector.tensor_mul(dv, dv, tri)
                    es_q[(t, qt)] = es
                if t >= LAG:
                    emit_pv(t - LAG)
                inject()
            for t in range(max(0, ntk - LAG), ntk):
                emit_pv(t)

            # drain strips to SBUF
            for qt in range(2):
                nc.vector.tensor_copy(xft[qt][:, j * 512:(j + 1) * 512],
                                      xfa[qt][:])

        # ---------------- tversky tail (per chunk) ----------------
        cc_in = [drpool.tile([16, 512], F32, name=f"ccin{j}") for j in range(NC)]
        cc_out = [drpool.tile([16, 512], F32, name=f"ccout{j}") for j in range(NC)]

        def tail_chunk_steps(j):
            s0 = j * 512
            st = {}

            def t1_dens():
                # gather dens rows (strip row 16 of each 32-block), then
                # reciprocal = exp(-ln) on ACT (set stays resident), stage
                # to DRAM for the partition-broadcast
                dens = tailp.tile([8, 512], F32, tag="dens", name="dens")
                dq = [nc.sync, nc.sync, nc.sync, nc.sync]
                for qt in range(2):
                    for p in range(4):
                        dq[p].dma_start(
                            dens[qt * 4 + p:qt * 4 + p + 1, :],
                            xft[qt][32 * p + 16:32 * p + 17, s0:s0 + 512])
                lnd = tailp.tile([8, 512], F32, tag="lnd", name="lnd")
                nc.scalar.activation(lnd[:], dens[:], AF.Ln)
                rb = tailp.tile([8, 512], F32, tag="rb", name="rb")
                nc.scalar.activation(rb[:], lnd[:], AF.Exp, scale=-1.0)
                drb = drpool.tile([8, 512], F32, tag="drb", bufs=2, name="drb")
                nc.sync.dma_start(drb[:], rb[:])
                st["drb"] = drb

            def t2_xf():
                drb = st["drb"]
                rbb = [tailp.tile([128, 512], F32, tag=f"rbb{qt}", name="rbb")
                       for qt in range(2)]
                dq = [nc.sync, nc.sync, nc.sync, nc.sync]
                for qt in range(2):
                    for p in range(4):
                        dq[p].dma_start(
                            rbb[qt][32 * p:32 * p + 16, :],
                            drb[qt * 4 + p:qt * 4 + p + 1, :]
                            .to_broadcast([16, 512]))
                sc = [tailp.tile([128, 512], BF16, tag=f"sc{qt}", name="sc")
                      for qt in range(2)]
                for qt in range(2):
                    nc.vector.tensor_mul(sc[qt][:],
                                         xft[qt][:, s0:s0 + 512], rbb[qt][:])
                pxf = aux_tile()
                for qt in range(2):
                    nc.tensor.matmul(pxf[0:16, :], smask[:], sc[qt][:],
                                     start=(qt == 0), stop=(qt == 1))
                xfl = tailp.tile([16, 512], F32, tag="xfl", name="xfl")
                nc.vector.tensor_copy(xfl[:], pxf[0:16, :])
                if dbg:
                    nc.sync.dma_start(d_dbg_xf[:, s0:s0 + 512], xfl[:])
                # write the collective input on the gpsimd queue: queue
                # order guarantees the write lands before the AllReduce reads
                # it (DRAM deps are untracked; the sync queue can be congested)
                nc.gpsimd.dma_start(cc_in[j][:], xfl[:])
                nc.gpsimd.collective_compute(
                    "AllReduce", ALU.add,
                    replica_groups=[[0, 1, 2, 3], [4, 5, 6, 7]],
                    ins=[cc_in[j][:]], outs=[cc_out[j][:]])

            def t3_out():
                xf = tailp.tile([16, 512], F32, tag="xfr", name="xfr")
                # read back on the gpsimd queue: the collective occupies that
                # queue, so queue order guarantees the AllReduce completed
                # (DRAM RAW deps are not tracked by the tile framework)
                nc.gpsimd.dma_start(xf[:], cc_out[j][:])
                if dbg:
                    nc.sync.dma_start(d_dbg_xfar[:, s0:s0 + 512], xf[:])
                # xa = xf*sig(5xf) = xf/(1+e), oms = e/(1+e), e = exp(-5 xf)
                e = tailp.tile([16, 512], F32, tag="e", name="e")
                nc.scalar.activation(e[:], xf[:], AF.Exp, scale=-5.0)
                t1 = tailp.tile([16, 512], F32, tag="t1", name="t1")
                nc.vector.tensor_scalar_add(t1[:], e[:], 1.0)
                lt = tailp.tile([16, 512], F32, tag="lt", name="lt")
                nc.scalar.activation(lt[:], t1[:], AF.Ln)
                r = tailp.tile([16, 512], F32, tag="r", name="r")
                nc.scalar.activation(r[:], lt[:], AF.Exp, scale=-1.0)
                xa = tailp.tile([16, 512], BF16, tag="xa", name="xa")
                oms = tailp.tile([16, 512], BF16, tag="oms", name="oms")
                nc.vector.tensor_mul(xa[:], xf[:], r[:])
                nc.vector.tensor_mul(oms[:], e[:], r[:])
                for ot in range(2):
                    po = aux_tile()
                    nc.tensor.matmul(po[:], AT[:, ot * 128:(ot + 1) * 128],
                                     xa[:], start=True, stop=False)
                    nc.tensor.matmul(po[:], BT[:, ot * 128:(ot + 1) * 128],
                                     oms[:], start=False, stop=True)
                    ob = tailp.tile([128, 512], F32, tag="ob", name="ob")
                    nc.vector.tensor_copy(ob[:], po[:])
                    nc.sync.dma_start(
                        d_out[ot * 128:(ot + 1) * 128, s0:s0 + 512], ob[:])

            return [t1_dens, t2_xf, t3_out]

        # ---------------- main schedule ----------------
        # attn(j) runs with proj(j+1) and tail(j-1) steps injected between
        # t-iterations so the PE never sits behind a long-latency tail chain.
        p0 = proj_chunk_steps(0)
        p0[0]()          # x loads
        p0[1]()          # first qk chain
        load_consts()
        for step in p0[2:]:
            step()
        held_t3 = []
        for j in range(NC):
            tail_steps = tail_chunk_steps(j - 1) if j >= 1 else [None] * 3
            if j >= NC - 2:
                # hold the last two chunks' post-collective outputs to fill
                # the final dens-chain and AllReduce latencies at the end
                held_t3.append(tail_steps[2])
                tail_steps = tail_steps[:2] + [None]
            proj_steps = (proj_chunk_steps(j + 1) if j + 1 < NC
                          else [None] * 8)
            # interleave: x + dens kickoff first, the collective (t2) early
            # enough to complete mid-chunk, the post-collective output (t3)
            # last so no PE op ever queues behind the AllReduce latency
            order = [proj_steps[0], tail_steps[0], proj_steps[1],
                     tail_steps[1], proj_steps[2], proj_steps[3],
                     proj_steps[4], proj_steps[5], proj_steps[6],
                     proj_steps[7], tail_steps[2]]
            pending = [s for s in order if s is not None]
            counter = [0]
            # spread injected steps over ~80% of the t loop
            horizon = max(1, int(4 * (j + 1) * 0.6))
            nsteps = len(pending)

            def inject():
                counter[0] += 1
                want = min(nsteps, -(-counter[0] * nsteps // horizon))
                while len(pending) and (nsteps - len(pending)) < want:
                    pending.pop(0)()
            attn_chunk(j, inject)
            while pending:
                pending.pop(0)()
        last = tail_chunk_steps(NC - 1)
        last[0]()
        held_t3[0]()         # chunk NC-3 output fills the dens-chain wait
        last[1]()            # final AllReduce kickoff
        held_t3[1]()         # chunk NC-2 output fills the AllReduce wait
        last[2]()

        persist.close()

    nc.compile()
    return nc


# ----------------------------------------------------------- host marshaling

def make_in_maps(S, x, w_qkv, features, prototypes, theta, alpha, beta,
                 q_gain, diff_lambda):
    x = np.asarray(x, np.float32)
    w_qkv = np.asarray(w_qkv, np.float32)
    features = np.asarray(features, np.float32)
    prototypes = np.asarray(prototypes, np.float32)
    theta = float(np.abs(np.asarray(theta, np.float32)))
    alpha = float(np.abs(np.asarray(alpha, np.float32)))
    beta = float(np.abs(np.asarray(beta, np.float32)))
    q_gain = np.asarray(q_gain, np.float32)
    lam = np.asarray(diff_lambda, np.float32)

    w_deq = _ternary_deq(w_qkv)
    p_deq = _ternary_deq(prototypes)
    cosT, sinT = _rope_tables(S)       # [32, S]

    rows = np.arange(128)
    sgn = np.where((rows // 32) % 2 == 0, 1.0, -1.0).astype(np.float32)
    cosF = cosT[rows % 32, :].astype(BF)
    sinF = (sinT[rows % 32, :] * sgn[:, None]).astype(BF)

    # diag-block causal mask, tiled 4x horizontally: [128, 512]
    tri = (np.arange(128)[None, :] >= np.arange(128)[:, None]).astype(np.float32)
    tri4 = np.tile(tri, (1, 4)).astype(BF)

    # stats masks: [128, 32] = 4 tile-types x 8 stat rows
    mask8 = np.zeros((128, 32), np.float32)
    for tt in range(4):
        for r in range(128):
            head_in_tile = r // 64
            row = (tt // 2) * 4 + (tt % 2) * 2 + head_in_tile
            mask8[r, tt * 8 + row] = 1.0
    mask8 = mask8.astype(BF)

    # strip-sum mask: rows 32p+i (i<16) -> col i
    smask = np.zeros((128, 16), np.float32)
    for p in range(4):
        for i in range(16):
            smask[32 * p + i, i] = 1.0

    ident = np.eye(128, dtype=np.float32).astype(BF)

    in_maps = []
    for c in range(N_CORES):
        b, hg = c // 4, c % 4
        h0 = hg * HPC
        qrows = slice(h0 * HD, (h0 + HPC) * HD)
        o0 = hg * OSL

        wq = w_deq[0:DIM][qrows]                   # [256, 1024]
        wk = w_deq[DIM:2 * DIM][qrows]             # [256, 1024]
        wv = w_deq[2 * DIM:3 * DIM][qrows]         # [256, 1024]

        # vw fold: per (h, f) channel block [k,16] = x @ (wv_f^T W_hf^T)
        # W_h0 = M1 + M2, W_h1 = lam_h (M2 - M1), M = features[:, head dims]
        wfold = np.zeros((DIM, 8 * NF), np.float32)
        for h in range(HPC):
            gh = h0 + h
            M1 = features[:, gh * 64:gh * 64 + 32]       # [16, 32]
            M2 = features[:, gh * 64 + 32:gh * 64 + 64]
            Wh0 = (M1 + M2)                               # [16, 32]
            Wh1 = lam[gh] * (M2 - M1)
            v0 = wv[h * 64:h * 64 + 32]                   # [32, 1024]
            v1 = wv[h * 64 + 32:h * 64 + 64]
            wfold[:, (2 * h) * NF:(2 * h + 1) * NF] = v0.T @ Wh0.T
            wfold[:, (2 * h + 1) * NF:(2 * h + 2) * NF] = v1.T @ Wh1.T

        wT = np.concatenate([wq.T, wk.T, wfold], axis=1)  # [1024, 640]

        gains = q_gain[h0:h0 + HPC] / math.sqrt(HD // 2)
        assert np.all(gains > 0), "nonpositive q_gain unsupported by ln-fold"
        lngain = np.zeros((8, 1), np.float32)
        lngain[0:4, 0] = np.log(gains)

        p_f = p_deq[o0:o0 + OSL] @ features.T          # [256, 16]
        p_s = _sigmoid(5.0 * p_f)
        p_a = p_f * p_s
        A_eff = theta * p_a - alpha * (1.0 - p_s)
        B_eff = -beta * p_a

        m = {
            "xT": np.ascontiguousarray(x[b].T).astype(BF),
            "wT": np.ascontiguousarray(wT).astype(BF),
            "cosF": cosF, "sinF": sinF,
            "tri4": tri4, "mask8": mask8,
            "smask": smask.astype(BF),
            "lngain": lngain,
            "ident": ident,
            "AT": np.ascontiguousarray(A_eff.T).astype(BF),
            "BT": np.ascontiguousarray(B_eff.T).astype(BF),
        }
        in_maps.append(m)
    return in_maps


def assemble_output(S, results):
    out = np.empty((B, S, DIM), np.float32)
    for c in range(N_CORES):
        b, hg = c // 4, c % 4
        out[b, :, hg * OSL:(hg + 1) * OSL] = results[c]["out"].T
    return out


_PROGRAM_CACHE = {}


def kernel(x, w_qkv, features, prototypes, theta, alpha, beta, q_gain,
           diff_lambda, _trace=False):
    x = np.asarray(x, np.float32)
    S = x.shape[1]
    if S not in _PROGRAM_CACHE:
        _PROGRAM_CACHE[S] = build_program(S)
    nc = _PROGRAM_CACHE[S]

    in_maps = make_in_maps(S, x, w_qkv, features, prototypes, theta, alpha,
                           beta, q_gain, diff_lambda)
    res = run_bass_kernel_spmd(nc, in_maps, list(range(N_CORES)),
                               trace=_trace)
    out = assemble_output(S, res.results)
    if _trace:
        return out, res
    return out
